# revision 8
# baseline (speedup 1.0000x reference)
"""ADD-S (symmetric) pose loss kernel for Trainium2, 8 NeuronCores.

Sharding: data-parallel over the batch dim B=8 -> one batch element per core.
Each core computes sum_n [ min_dist(n) * conf(n) - W*log(conf(n)) ] for its
4096 points, returned as [128,1] per-partition partial sums; the host sums the
8*128 partials and divides by B*N.

Device algorithm (per core, N = 4096 points):
  1. Elementwise prologue on DVE in a SoA layout ([128 partitions, 32 free],
     point n lives at (p, f) = (n >> 5, n & 31)):
       - quat -> rotation via the unnormalized form R = M / |q|^2
       - points_model = R_gt @ (points - t_gt)      (per-batch scalars
         broadcast to [128,1] columns, applied with scalar_tensor_tensor)
       - points_pred  = R_pred @ points_model + trans
       - aa = |points_pred|^2, bb = |points|^2
  2. Build matmul operands (SBUF->SBUF flatten DMAs, identity order
     n = p*32 + f):  lhsT [5, 4096] = [aa, 1, -2*pp_x, -2*pp_y, -2*pp_z],
     rhs [5, 4096] = [1, bb, p_x, p_y, p_z].
  3. d2[n, m] = aa[n] + bb[m] - 2*<pp_n, q_m> as K=5 matmuls on the PE:
     32 n-blocks x 8 m-tiles of [128, 512] f32 into PSUM.
  4. min over m on DVE: reduce_min over [128, 2048] 4-bank PSUM spans
     (2 per n-block, double buffered against the PE).
  5. dist = sqrt(max(min_d2, 1e-12)); pixel = dist*clip(conf) - W*ln(clip(conf));
     per-partition row sums -> [128, 1] output.
"""

import numpy as np

B = 8
N = 4096
P = 128
F = N // P          # 32 free elems per partition in SoA layout
NB = N // P         # 32 n-blocks of 128
MSPAN = 2048        # PSUM reduce span (4 banks)
W_RATE = 0.015
SYM_CLASS_IDS = {1}

_cache = {}


def _np_f32(x):
    return np.ascontiguousarray(np.asarray(x), dtype=np.float32)


def _emit(ctx, tc, out_ap, ins):
    import concourse.bass as bass
    from concourse import mybir

    nc = tc.nc
    f32 = mybir.dt.float32
    Alu = mybir.AluOpType
    Act = mybir.ActivationFunctionType
    X = mybir.AxisListType.X

    quat, trans, conf, pose, points = (
        ins["pred_quat"], ins["pred_trans"], ins["pred_conf"],
        ins["pose"], ins["points"],
    )

    pool = ctx.enter_context(tc.tile_pool(name="main", bufs=1))

    def t(tag, shape, dtype=f32):
        return pool.tile(shape, dtype, tag=tag, name=tag)

    dma = nc.sync.dma_start

    # ---------------- input loads ----------------
    q_t = t("q_t", [P, F * 4])       # quat rows, 4 per point
    p_t = t("p_t", [P, F * 3])       # points
    tr_t = t("tr_t", [P, F * 3])     # pred_trans
    bc = t("bc", [P, 12])            # pose scalars broadcast across partitions
    conf_b = t("conf_b", [P, NB])    # conf in output (SoA-B) order

    nc.gpsimd.dma_start(out=q_t, in_=quat.rearrange("(p f) c -> p (f c)", p=P))
    nc.gpsimd.dma_start(out=p_t, in_=points.rearrange("(p f) c -> p (f c)", p=P))
    nc.gpsimd.dma_start(out=tr_t, in_=trans.rearrange("(p f) c -> p (f c)", p=P))
    nc.gpsimd.dma_start(out=bc, in_=bass.AP(tensor=pose.tensor,
                                            offset=pose.offset,
                                            ap=[[0, P], [1, 12]]))
    # conf[b*128 + p] -> conf_b[p, b]  (strided gather, overlapped under loop)
    nc.gpsimd.dma_start(out=conf_b, in_=bass.AP(tensor=conf.tensor,
                                                offset=conf.offset,
                                                ap=[[1, P], [P, NB]]))

    q3 = q_t.rearrange("p (f c) -> p f c", c=4)
    p3 = p_t.rearrange("p (f c) -> p f c", c=3)
    tr3 = tr_t.rearrange("p (f c) -> p f c", c=3)

    vec = nc.vector

    # DMA-wait funnel: a chain of TT ops absorbs every input-DMA semaphore
    # wait (1 per instruction) so downstream TensorScalar ops, which have
    # very few HW sync-wait slots, never carry DMA waits themselves.  All
    # early DVE consumers of DMA'd tiles are order-pinned after the funnel.
    from concourse.tile import add_dep_helper

    scrf = t("scrf", [P, 1])
    vec.tensor_copy(out=scrf, in_=q_t[:, 0:1])
    for dep_t in (p_t, tr_t, bc, conf_b):
        last_f = vec.tensor_tensor(out=scrf, in0=scrf, in1=dep_t[:, 0:1],
                                   op=Alu.add)

    def pin(inst):
        add_dep_helper(inst.ins, last_f.ins, sync=False,
                       reason="order after input-DMA funnel")
        return inst

    # ---------------- conf term (early: ACT Ln table load overlaps) -------
    cc = t("cc", [P, NB])
    pin(vec.tensor_scalar_max(cc, conf_b, 1e-4))
    vec.tensor_scalar_min(cc, cc, 1.0)
    lnc = t("lnc", [P, NB])
    nc.scalar.activation(lnc, cc, Act.Ln)

    # ---------------- quaternion -> unnormalized rotation ----------------
    sq = t("sq", [P, F * 4])
    pin(vec.tensor_tensor(out=sq, in0=q_t, in1=q_t, op=Alu.mult))
    sq3 = sq.rearrange("p (f c) -> p f c", c=4)
    s2 = t("s2", [P, F])
    vec.reduce_sum(s2, sq3, axis=X)
    s2c = t("s2c", [P, F])
    vec.tensor_scalar_max(s2c, s2, 1e-16)
    rec = t("rec", [P, F])
    vec.reciprocal(rec, s2c)

    # gt transform: pm_k = sum_j Rg[k,j] * (points_j - t_j)
    # pose flat layout: Rg[k][j] = bc[:, 4k+j], t[j] = bc[:, 4j+3]
    pc = [t(f"pc{j}", [P, F]) for j in range(3)]
    for j in range(3):
        pin(vec.tensor_scalar_sub(pc[j], p3[:, :, j],
                                  bc[:, 4 * j + 3: 4 * j + 4]))
    pm = [t(f"pm{k}", [P, F]) for k in range(3)]
    for k in range(3):
        pin(vec.tensor_scalar_mul(pm[k], pc[0], bc[:, 4 * k: 4 * k + 1]))
        vec.scalar_tensor_tensor(out=pm[k], in0=pc[1],
                                 scalar=bc[:, 4 * k + 1: 4 * k + 2],
                                 in1=pm[k], op0=Alu.mult, op1=Alu.add)
        vec.scalar_tensor_tensor(out=pm[k], in0=pc[2],
                                 scalar=bc[:, 4 * k + 2: 4 * k + 3],
                                 in1=pm[k], op0=Alu.mult, op1=Alu.add)

    qw, qx, qy, qz = (q3[:, :, 0], q3[:, :, 1], q3[:, :, 2], q3[:, :, 3])
    xx, yy, zz = (sq3[:, :, 1], sq3[:, :, 2], sq3[:, :, 3])

    def dbl_prod(tag, a, b_):
        o = t(tag, [P, F])
        pin(vec.scalar_tensor_tensor(out=o, in0=a, scalar=2.0, in1=b_,
                                     op0=Alu.mult, op1=Alu.mult))
        return o

    xy2 = dbl_prod("xy2", qx, qy)
    xz2 = dbl_prod("xz2", qx, qz)
    yz2 = dbl_prod("yz2", qy, qz)
    wx2 = dbl_prod("wx2", qw, qx)
    wy2 = dbl_prod("wy2", qw, qy)
    wz2 = dbl_prod("wz2", qw, qz)

    def tt(tag, a, b_, op):
        o = t(tag, [P, F])
        vec.tensor_tensor(out=o, in0=a, in1=b_, op=op)
        return o

    b01m = tt("b01m", xy2, wz2, Alu.subtract)   # M[0][1]
    b01p = tt("b01p", xy2, wz2, Alu.add)        # M[1][0]
    b02p = tt("b02p", xz2, wy2, Alu.add)        # M[0][2]
    b02m = tt("b02m", xz2, wy2, Alu.subtract)   # M[2][0]
    b12m = tt("b12m", yz2, wx2, Alu.subtract)   # M[1][2]
    b12p = tt("b12p", yz2, wx2, Alu.add)        # M[2][1]

    a0 = tt("a0", yy, zz, Alu.add)
    a1 = tt("a1", xx, zz, Alu.add)
    a2 = tt("a2", xx, yy, Alu.add)
    u = []
    for k, ak in enumerate((a0, a1, a2)):
        uk = t(f"u{k}", [P, F])
        vec.scalar_tensor_tensor(out=uk, in0=ak, scalar=-2.0, in1=s2,
                                 op0=Alu.mult, op1=Alu.add)
        u.append(uk)

    # M rows (unnormalized R * s2):
    #   row0: [u0,   b01m, b02p]
    #   row1: [b01p, u1,   b12m]
    #   row2: [b02m, b12p, u2 ]
    rows = [(u[0], b01m, b02p), (b01p, u[1], b12m), (b02m, b12p, u[2])]
    pp = []
    scr = t("scr", [P, F])
    for i, (m0, m1, m2) in enumerate(rows):
        v = t(f"v{i}", [P, F])
        vec.tensor_tensor(out=v, in0=m0, in1=pm[0], op=Alu.mult)
        vec.tensor_tensor(out=scr, in0=m1, in1=pm[1], op=Alu.mult)
        vec.tensor_tensor(out=v, in0=v, in1=scr, op=Alu.add)
        vec.tensor_tensor(out=scr, in0=m2, in1=pm[2], op=Alu.mult)
        vec.tensor_tensor(out=v, in0=v, in1=scr, op=Alu.add)
        # pp_i = v * rec + trans_i
        vec.tensor_tensor(out=v, in0=v, in1=rec, op=Alu.mult)
        vec.tensor_tensor(out=v, in0=v, in1=tr3[:, :, i], op=Alu.add)
        pp.append(v)

    aa = t("aa", [P, F])
    vec.tensor_tensor(out=aa, in0=pp[0], in1=pp[0], op=Alu.mult)
    vec.tensor_tensor(out=scr, in0=pp[1], in1=pp[1], op=Alu.mult)
    vec.tensor_tensor(out=aa, in0=aa, in1=scr, op=Alu.add)
    vec.tensor_tensor(out=scr, in0=pp[2], in1=pp[2], op=Alu.mult)
    vec.tensor_tensor(out=aa, in0=aa, in1=scr, op=Alu.add)

    n2 = []
    for i in range(3):
        o = t(f"n2_{i}", [P, F])
        vec.tensor_scalar_mul(o, pp[i], -2.0)
        n2.append(o)

    # bb = |points|^2 ; compact copies of point coords for the flatten
    sqp = t("sqp", [P, F * 3])
    pin(vec.tensor_tensor(out=sqp, in0=p_t, in1=p_t, op=Alu.mult))
    bb = t("bb", [P, F])
    vec.reduce_sum(bb, sqp.rearrange("p (f c) -> p f c", c=3), axis=X)
    pcomp = []
    for j in range(3):
        o = t(f"pcomp{j}", [P, F])
        pin(vec.tensor_copy(out=o, in_=p3[:, :, j]))
        pcomp.append(o)

    ones_t = t("ones_t", [P, F])
    vec.memset(ones_t, 1.0)

    # ---------------- matmul operands ----------------
    lhsT = t("lhsT", [5, N])
    rhs = t("rhs", [5, N])
    # identity-order flatten: [128, 32] -> [1, 4096] with n = p*32 + f
    dma(out=lhsT[0:1, :], in_=aa)
    dma(out=lhsT[1:2, :], in_=ones_t)
    for i in range(3):
        dma(out=lhsT[2 + i: 3 + i, :], in_=n2[i])
    dma(out=rhs[0:1, :], in_=ones_t)
    dma(out=rhs[1:2, :], in_=bb)
    for j in range(3):
        dma(out=rhs[2 + j: 3 + j, :], in_=pcomp[j])

    # ---------------- main loop: d2 matmul + min over m ----------------
    mins2 = t("mins2", [P, 2 * NB])
    pp_psum = ctx.enter_context(tc.tile_pool(name="d2p", bufs=2, space="PSUM"))
    for b in range(NB):
        for half in range(2):
            ps = pp_psum.tile([P, MSPAN], f32, tag="d2")
            for j in range(MSPAN // 512):
                m0 = half * MSPAN + j * 512
                nc.tensor.matmul(
                    ps[:, j * 512:(j + 1) * 512],
                    lhsT[:, b * P:(b + 1) * P],
                    rhs[:, m0:m0 + 512],
                    start=True, stop=True,
                )
            col = 2 * b + half
            vec.tensor_reduce(mins2[:, col:col + 1], ps, axis=X, op=Alu.min)

    # ---------------- tail: combine, sqrt, pixel loss, row sums ----------
    m2v = mins2.rearrange("p (b two) -> p b two", two=2)
    minsb = t("minsb", [P, NB])
    vec.tensor_tensor(out=minsb, in0=m2v[:, :, 0], in1=m2v[:, :, 1], op=Alu.min)
    vec.tensor_scalar_max(minsb, minsb, 1e-12)
    dist = t("dist", [P, NB])
    nc.scalar.sqrt(dist, minsb)

    pix = t("pix", [P, NB])
    vec.tensor_tensor(out=pix, in0=dist, in1=cc, op=Alu.mult)
    vec.scalar_tensor_tensor(out=pix, in0=lnc, scalar=-W_RATE, in1=pix,
                             op0=Alu.mult, op1=Alu.add)
    sums = t("sums", [P, 1])
    vec.reduce_sum(sums, pix, axis=X)
    dma(out=out_ap, in_=sums)


def _build():
    from contextlib import ExitStack

    import concourse.bacc as bacc
    import concourse.tile as tile
    from concourse import mybir

    f32 = mybir.dt.float32
    nc = bacc.Bacc("TRN2", debug=False, enable_asserts=False, num_devices=B)
    ins = {
        "pred_quat": nc.dram_tensor("pred_quat", [N, 4], f32,
                                    kind="ExternalInput").ap(),
        "pred_trans": nc.dram_tensor("pred_trans", [N, 3], f32,
                                     kind="ExternalInput").ap(),
        "pred_conf": nc.dram_tensor("pred_conf", [N, 1], f32,
                                    kind="ExternalInput").ap(),
        "pose": nc.dram_tensor("pose", [3, 4], f32, kind="ExternalInput").ap(),
        "points": nc.dram_tensor("points", [N, 3], f32,
                                 kind="ExternalInput").ap(),
    }
    out_ap = nc.dram_tensor("out_sums", [P, 1], f32, kind="ExternalOutput").ap()
    with tile.TileContext(nc) as tc:
        with ExitStack() as ctx:
            _emit(ctx, tc, out_ap, ins)
    nc.compile()
    return nc


def _get_nc():
    if "nc" not in _cache:
        _cache["nc"] = _build()
    return _cache["nc"]


def _numpy_reference(pred_quat, pred_trans, pred_conf, pose, points, cls_id):
    """Full-precision numpy fallback (used only for the non-symmetric branch)."""
    q = pred_quat.astype(np.float64)
    q = q / np.clip(np.linalg.norm(q, axis=-1, keepdims=True), 1e-8, None)
    w, x, y, z = q[..., 0], q[..., 1], q[..., 2], q[..., 3]
    r = np.stack([
        1 - 2 * (y * y + z * z), 2 * (x * y - w * z), 2 * (x * z + w * y),
        2 * (x * y + w * z), 1 - 2 * (x * x + z * z), 2 * (y * z - w * x),
        2 * (x * z - w * y), 2 * (y * z + w * x), 1 - 2 * (x * x + y * y),
    ], axis=-1).reshape(q.shape[:-1] + (3, 3))
    gt_r = pose[:, :3, :3].astype(np.float64)
    gt_t = pose[:, :3, 3].astype(np.float64)
    pc = points.astype(np.float64) - gt_t[:, None, :]
    pm = np.einsum("bkj,bnj->bnk", gt_r, pc)
    ppred = np.einsum("bnij,bnj->bni", r, pm) + pred_trans.astype(np.float64)
    tgt = points.astype(np.float64)
    if int(cls_id[0]) in SYM_CLASS_IDS:
        aa = np.sum(ppred * ppred, axis=-1)
        bb2 = np.sum(tgt * tgt, axis=-1)
        ab = np.einsum("bnd,bmd->bnm", ppred, tgt)
        d2 = aa[:, :, None] + bb2[:, None, :] - 2.0 * ab
        loss_dist = np.sqrt(np.maximum(d2, 1e-12)).min(axis=2)
    else:
        loss_dist = np.linalg.norm(ppred - tgt, axis=2)
    c = np.clip(pred_conf[..., 0].astype(np.float64), 1e-4, 1.0)
    return np.float32(np.mean(loss_dist * c - W_RATE * np.log(c)))


def kernel(pred_quat, pred_trans, pred_conf, pose, points, cls_id):
    pred_quat = _np_f32(pred_quat)
    pred_trans = _np_f32(pred_trans)
    pred_conf = _np_f32(pred_conf)
    pose = _np_f32(pose)
    points = _np_f32(points)
    cls_id = np.asarray(cls_id)

    assert pred_quat.shape == (B, N, 4), pred_quat.shape

    if int(cls_id[0]) not in SYM_CLASS_IDS:
        return np.array(
            _numpy_reference(pred_quat, pred_trans, pred_conf, pose, points,
                             cls_id),
            dtype=np.float32)

    from concourse.bass_utils import run_bass_kernel_spmd

    nc = _get_nc()
    in_maps = [
        {
            "pred_quat": np.ascontiguousarray(pred_quat[c]),
            "pred_trans": np.ascontiguousarray(pred_trans[c]),
            "pred_conf": np.ascontiguousarray(pred_conf[c]),
            "pose": np.ascontiguousarray(pose[c]),
            "points": np.ascontiguousarray(points[c]),
        }
        for c in range(B)
    ]
    res = run_bass_kernel_spmd(nc, in_maps, core_ids=list(range(B)))
    total = np.float64(0.0)
    for r in res.results:
        total += np.sum(r["out_sums"].astype(np.float64))
    return np.array(total / (B * N), dtype=np.float32)


# revision 9
# speedup vs baseline: 2.5624x; 2.5624x over previous
"""ADD-S (symmetric) pose loss kernel for Trainium2, 8 NeuronCores.

Sharding: data-parallel over the batch dim B=8 -> one batch element per core.
Each core computes sum_n [ min_dist(n) * conf(n) - W*log(conf(n)) ] for its
4096 points, returned as [128,1] per-partition partial sums; the host sums the
8*128 partials and divides by B*N.

Device algorithm (per core, N = 4096 points):
  1. Elementwise prologue on DVE in a SoA layout ([128 partitions, 32 free],
     point n lives at (p, f) = (n >> 5, n & 31)):
       - quat -> rotation via the unnormalized form R = M / |q|^2
       - points_model = R_gt @ (points - t_gt)      (per-batch scalars
         broadcast to [128,1] columns, applied with scalar_tensor_tensor)
       - points_pred  = R_pred @ points_model + trans
       - aa = |points_pred|^2, bb = |points|^2
  2. Quantize coordinates to bf16 (pp~, q~) and compute aa=|pp~|^2,
     bb=|q~|^2 in f32 FROM the quantized coords, carried as exact bf16
     hi+lo row pairs.  d2 = |pp~ - q~|^2 is then computed exactly for the
     perturbed points -- the bf16 error acts as a tiny zero-mean point
     perturbation instead of a catastrophic cancellation error.
     Operands (SBUF->SBUF flatten DMAs, identity order n = p*32 + f):
       lhsT [7, 4096] bf16 = [aa_h, aa_l, 1, 1, -2pp~_x, -2pp~_y, -2pp~_z]
       rhs  [7, 4096] bf16 = [1,    1, bb_h, bb_l,  q~_x,   q~_y,   q~_z]
  3. d2[n, m] as single-pass bf16 K=7 matmuls on the PE:
     32 n-blocks x 8 m-tiles of [128, 512] f32 into PSUM.
  4. min over m on DVE: reduce_min over [128, 2048] 4-bank PSUM spans
     (2 per n-block, double buffered against the PE).
  5. dist = sqrt(max(min_d2, 1e-12)); pixel = dist*clip(conf) - W*ln(clip(conf));
     per-partition row sums -> [128, 1] output.
"""

import numpy as np

B = 8
N = 4096
P = 128
F = N // P          # 32 free elems per partition in SoA layout
NB = N // P         # 32 n-blocks of 128
MSPAN = 2048        # PSUM reduce span (4 banks)
W_RATE = 0.015
SYM_CLASS_IDS = {1}

_cache = {}


def _np_f32(x):
    return np.ascontiguousarray(np.asarray(x), dtype=np.float32)


def _emit(ctx, tc, out_ap, ins):
    import concourse.bass as bass
    from concourse import mybir

    nc = tc.nc
    f32 = mybir.dt.float32
    Alu = mybir.AluOpType
    Act = mybir.ActivationFunctionType
    X = mybir.AxisListType.X

    quat, trans, conf, pose, points = (
        ins["pred_quat"], ins["pred_trans"], ins["pred_conf"],
        ins["pose"], ins["points"],
    )

    pool = ctx.enter_context(tc.tile_pool(name="main", bufs=1))

    def t(tag, shape, dtype=f32):
        return pool.tile(shape, dtype, tag=tag, name=tag)

    dma = nc.sync.dma_start

    # ---------------- input loads ----------------
    q_t = t("q_t", [P, F * 4])       # quat rows, 4 per point
    p_t = t("p_t", [P, F * 3])       # points
    tr_t = t("tr_t", [P, F * 3])     # pred_trans
    bc = t("bc", [P, 12])            # pose scalars broadcast across partitions
    conf_b = t("conf_b", [P, NB])    # conf in output (SoA-B) order

    nc.gpsimd.dma_start(out=q_t, in_=quat.rearrange("(p f) c -> p (f c)", p=P))
    nc.gpsimd.dma_start(out=p_t, in_=points.rearrange("(p f) c -> p (f c)", p=P))
    nc.gpsimd.dma_start(out=tr_t, in_=trans.rearrange("(p f) c -> p (f c)", p=P))
    nc.gpsimd.dma_start(out=bc, in_=bass.AP(tensor=pose.tensor,
                                            offset=pose.offset,
                                            ap=[[0, P], [1, 12]]))
    # conf[b*128 + p] -> conf_b[p, b]  (strided gather, overlapped under loop)
    nc.gpsimd.dma_start(out=conf_b, in_=bass.AP(tensor=conf.tensor,
                                                offset=conf.offset,
                                                ap=[[1, P], [P, NB]]))

    q3 = q_t.rearrange("p (f c) -> p f c", c=4)
    p3 = p_t.rearrange("p (f c) -> p f c", c=3)
    tr3 = tr_t.rearrange("p (f c) -> p f c", c=3)

    vec = nc.vector

    # DMA-wait funnel: a chain of TT ops absorbs every input-DMA semaphore
    # wait (1 per instruction) so downstream TensorScalar ops, which have
    # very few HW sync-wait slots, never carry DMA waits themselves.  All
    # early DVE consumers of DMA'd tiles are order-pinned after the funnel.
    from concourse.tile import add_dep_helper

    scrf = t("scrf", [P, 1])
    vec.tensor_copy(out=scrf, in_=q_t[:, 0:1])
    for dep_t in (p_t, tr_t, bc, conf_b):
        last_f = vec.tensor_tensor(out=scrf, in0=scrf, in1=dep_t[:, 0:1],
                                   op=Alu.add)

    def pin(inst):
        add_dep_helper(inst.ins, last_f.ins, sync=False,
                       reason="order after input-DMA funnel")
        return inst

    # ---------------- conf term (early: ACT Ln table load overlaps) -------
    cc = t("cc", [P, NB])
    pin(vec.tensor_scalar_max(cc, conf_b, 1e-4))
    vec.tensor_scalar_min(cc, cc, 1.0)
    lnc = t("lnc", [P, NB])
    nc.scalar.activation(lnc, cc, Act.Ln)

    # ---------------- quaternion -> unnormalized rotation ----------------
    sq = t("sq", [P, F * 4])
    pin(vec.tensor_tensor(out=sq, in0=q_t, in1=q_t, op=Alu.mult))
    sq3 = sq.rearrange("p (f c) -> p f c", c=4)
    s2 = t("s2", [P, F])
    vec.reduce_sum(s2, sq3, axis=X)
    s2c = t("s2c", [P, F])
    vec.tensor_scalar_max(s2c, s2, 1e-16)
    rec = t("rec", [P, F])
    vec.reciprocal(rec, s2c)

    # gt transform: pm_k = sum_j Rg[k,j] * (points_j - t_j)
    # pose flat layout: Rg[k][j] = bc[:, 4k+j], t[j] = bc[:, 4j+3]
    pc = [t(f"pc{j}", [P, F]) for j in range(3)]
    for j in range(3):
        pin(vec.tensor_scalar_sub(pc[j], p3[:, :, j],
                                  bc[:, 4 * j + 3: 4 * j + 4]))
    pm = [t(f"pm{k}", [P, F]) for k in range(3)]
    for k in range(3):
        pin(vec.tensor_scalar_mul(pm[k], pc[0], bc[:, 4 * k: 4 * k + 1]))
        vec.scalar_tensor_tensor(out=pm[k], in0=pc[1],
                                 scalar=bc[:, 4 * k + 1: 4 * k + 2],
                                 in1=pm[k], op0=Alu.mult, op1=Alu.add)
        vec.scalar_tensor_tensor(out=pm[k], in0=pc[2],
                                 scalar=bc[:, 4 * k + 2: 4 * k + 3],
                                 in1=pm[k], op0=Alu.mult, op1=Alu.add)

    qw, qx, qy, qz = (q3[:, :, 0], q3[:, :, 1], q3[:, :, 2], q3[:, :, 3])
    xx, yy, zz = (sq3[:, :, 1], sq3[:, :, 2], sq3[:, :, 3])

    def dbl_prod(tag, a, b_):
        o = t(tag, [P, F])
        pin(vec.scalar_tensor_tensor(out=o, in0=a, scalar=2.0, in1=b_,
                                     op0=Alu.mult, op1=Alu.mult))
        return o

    xy2 = dbl_prod("xy2", qx, qy)
    xz2 = dbl_prod("xz2", qx, qz)
    yz2 = dbl_prod("yz2", qy, qz)
    wx2 = dbl_prod("wx2", qw, qx)
    wy2 = dbl_prod("wy2", qw, qy)
    wz2 = dbl_prod("wz2", qw, qz)

    def tt(tag, a, b_, op):
        o = t(tag, [P, F])
        vec.tensor_tensor(out=o, in0=a, in1=b_, op=op)
        return o

    b01m = tt("b01m", xy2, wz2, Alu.subtract)   # M[0][1]
    b01p = tt("b01p", xy2, wz2, Alu.add)        # M[1][0]
    b02p = tt("b02p", xz2, wy2, Alu.add)        # M[0][2]
    b02m = tt("b02m", xz2, wy2, Alu.subtract)   # M[2][0]
    b12m = tt("b12m", yz2, wx2, Alu.subtract)   # M[1][2]
    b12p = tt("b12p", yz2, wx2, Alu.add)        # M[2][1]

    a0 = tt("a0", yy, zz, Alu.add)
    a1 = tt("a1", xx, zz, Alu.add)
    a2 = tt("a2", xx, yy, Alu.add)
    u = []
    for k, ak in enumerate((a0, a1, a2)):
        uk = t(f"u{k}", [P, F])
        vec.scalar_tensor_tensor(out=uk, in0=ak, scalar=-2.0, in1=s2,
                                 op0=Alu.mult, op1=Alu.add)
        u.append(uk)

    # M rows (unnormalized R * s2):
    #   row0: [u0,   b01m, b02p]
    #   row1: [b01p, u1,   b12m]
    #   row2: [b02m, b12p, u2 ]
    rows = [(u[0], b01m, b02p), (b01p, u[1], b12m), (b02m, b12p, u[2])]
    pp = []
    scr = t("scr", [P, F])
    for i, (m0, m1, m2) in enumerate(rows):
        v = t(f"v{i}", [P, F])
        vec.tensor_tensor(out=v, in0=m0, in1=pm[0], op=Alu.mult)
        vec.tensor_tensor(out=scr, in0=m1, in1=pm[1], op=Alu.mult)
        vec.tensor_tensor(out=v, in0=v, in1=scr, op=Alu.add)
        vec.tensor_tensor(out=scr, in0=m2, in1=pm[2], op=Alu.mult)
        vec.tensor_tensor(out=v, in0=v, in1=scr, op=Alu.add)
        # pp_i = v * rec + trans_i
        vec.tensor_tensor(out=v, in0=v, in1=rec, op=Alu.mult)
        vec.tensor_tensor(out=v, in0=v, in1=tr3[:, :, i], op=Alu.add)
        pp.append(v)

    bf16 = mybir.dt.bfloat16

    # quantize predicted points to bf16; aa computed in f32 FROM the
    # quantized coords, then split into an exact bf16 hi+lo pair
    ppq, n2 = [], []
    for i in range(3):
        q_ = t(f"ppq{i}", [P, F], bf16)
        vec.tensor_copy(out=q_, in_=pp[i])
        ppq.append(q_)
        o = t(f"n2_{i}", [P, F], bf16)
        vec.tensor_scalar_mul(o, q_, -2.0)     # exact: *2 and bf16 input
        n2.append(o)
    aa = t("aa", [P, F])
    vec.tensor_tensor(out=aa, in0=ppq[0], in1=ppq[0], op=Alu.mult)
    vec.tensor_tensor(out=scr, in0=ppq[1], in1=ppq[1], op=Alu.mult)
    vec.tensor_tensor(out=aa, in0=aa, in1=scr, op=Alu.add)
    vec.tensor_tensor(out=scr, in0=ppq[2], in1=ppq[2], op=Alu.mult)
    vec.tensor_tensor(out=aa, in0=aa, in1=scr, op=Alu.add)
    aa_h = t("aa_h", [P, F], bf16)
    vec.tensor_copy(out=aa_h, in_=aa)
    aa_l = t("aa_l", [P, F], bf16)
    vec.tensor_tensor(out=aa_l, in0=aa, in1=aa_h, op=Alu.subtract)

    # quantized target coords + bb = |q~|^2 as exact hi+lo pair
    pcomp = []
    for j in range(3):
        o = t(f"pcomp{j}", [P, F], bf16)
        pin(vec.tensor_copy(out=o, in_=p3[:, :, j]))
        pcomp.append(o)
    bb = t("bb", [P, F])
    vec.tensor_tensor(out=bb, in0=pcomp[0], in1=pcomp[0], op=Alu.mult)
    vec.tensor_tensor(out=scr, in0=pcomp[1], in1=pcomp[1], op=Alu.mult)
    vec.tensor_tensor(out=bb, in0=bb, in1=scr, op=Alu.add)
    vec.tensor_tensor(out=scr, in0=pcomp[2], in1=pcomp[2], op=Alu.mult)
    vec.tensor_tensor(out=bb, in0=bb, in1=scr, op=Alu.add)
    bb_h = t("bb_h", [P, F], bf16)
    vec.tensor_copy(out=bb_h, in_=bb)
    bb_l = t("bb_l", [P, F], bf16)
    vec.tensor_tensor(out=bb_l, in0=bb, in1=bb_h, op=Alu.subtract)

    ones_t = t("ones_t", [P, F], bf16)
    vec.memset(ones_t, 1.0)

    # ---------------- matmul operands ----------------
    K_DIM = 7
    lhsT = t("lhsT", [K_DIM, N], bf16)
    rhs = t("rhs", [K_DIM, N], bf16)
    # identity-order flatten: [128, 32] -> [1, 4096] with n = p*32 + f
    for i, row in enumerate((aa_h, aa_l, ones_t, ones_t, n2[0], n2[1], n2[2])):
        dma(out=lhsT[i:i + 1, :], in_=row)
    for i, row in enumerate((ones_t, ones_t, bb_h, bb_l,
                             pcomp[0], pcomp[1], pcomp[2])):
        dma(out=rhs[i:i + 1, :], in_=row)

    # ---------------- main loop: d2 matmul + min over m ----------------
    mins2 = t("mins2", [P, 2 * NB])
    pp_psum = ctx.enter_context(tc.tile_pool(name="d2p", bufs=2, space="PSUM"))
    for b in range(NB):
        for half in range(2):
            ps = pp_psum.tile([P, MSPAN], f32, tag="d2")
            for j in range(MSPAN // 512):
                m0 = half * MSPAN + j * 512
                nc.tensor.matmul(
                    ps[:, j * 512:(j + 1) * 512],
                    lhsT[:, b * P:(b + 1) * P],
                    rhs[:, m0:m0 + 512],
                    start=True, stop=True,
                )
            col = 2 * b + half
            vec.tensor_reduce(mins2[:, col:col + 1], ps, axis=X, op=Alu.min)

    # ---------------- tail: combine, sqrt, pixel loss, row sums ----------
    m2v = mins2.rearrange("p (b two) -> p b two", two=2)
    minsb = t("minsb", [P, NB])
    vec.tensor_tensor(out=minsb, in0=m2v[:, :, 0], in1=m2v[:, :, 1], op=Alu.min)
    vec.tensor_scalar_max(minsb, minsb, 1e-12)
    dist = t("dist", [P, NB])
    nc.scalar.sqrt(dist, minsb)

    pix = t("pix", [P, NB])
    vec.tensor_tensor(out=pix, in0=dist, in1=cc, op=Alu.mult)
    vec.scalar_tensor_tensor(out=pix, in0=lnc, scalar=-W_RATE, in1=pix,
                             op0=Alu.mult, op1=Alu.add)
    sums = t("sums", [P, 1])
    vec.reduce_sum(sums, pix, axis=X)
    dma(out=out_ap, in_=sums)


def _build():
    from contextlib import ExitStack

    import concourse.bacc as bacc
    import concourse.tile as tile
    from concourse import mybir

    f32 = mybir.dt.float32
    nc = bacc.Bacc("TRN2", debug=False, enable_asserts=False, num_devices=B)
    ins = {
        "pred_quat": nc.dram_tensor("pred_quat", [N, 4], f32,
                                    kind="ExternalInput").ap(),
        "pred_trans": nc.dram_tensor("pred_trans", [N, 3], f32,
                                     kind="ExternalInput").ap(),
        "pred_conf": nc.dram_tensor("pred_conf", [N, 1], f32,
                                    kind="ExternalInput").ap(),
        "pose": nc.dram_tensor("pose", [3, 4], f32, kind="ExternalInput").ap(),
        "points": nc.dram_tensor("points", [N, 3], f32,
                                 kind="ExternalInput").ap(),
    }
    out_ap = nc.dram_tensor("out_sums", [P, 1], f32, kind="ExternalOutput").ap()
    with tile.TileContext(nc) as tc:
        with ExitStack() as ctx:
            _emit(ctx, tc, out_ap, ins)
    nc.compile()
    return nc


def _get_nc():
    if "nc" not in _cache:
        _cache["nc"] = _build()
    return _cache["nc"]


def _numpy_reference(pred_quat, pred_trans, pred_conf, pose, points, cls_id):
    """Full-precision numpy fallback (used only for the non-symmetric branch)."""
    q = pred_quat.astype(np.float64)
    q = q / np.clip(np.linalg.norm(q, axis=-1, keepdims=True), 1e-8, None)
    w, x, y, z = q[..., 0], q[..., 1], q[..., 2], q[..., 3]
    r = np.stack([
        1 - 2 * (y * y + z * z), 2 * (x * y - w * z), 2 * (x * z + w * y),
        2 * (x * y + w * z), 1 - 2 * (x * x + z * z), 2 * (y * z - w * x),
        2 * (x * z - w * y), 2 * (y * z + w * x), 1 - 2 * (x * x + y * y),
    ], axis=-1).reshape(q.shape[:-1] + (3, 3))
    gt_r = pose[:, :3, :3].astype(np.float64)
    gt_t = pose[:, :3, 3].astype(np.float64)
    pc = points.astype(np.float64) - gt_t[:, None, :]
    pm = np.einsum("bkj,bnj->bnk", gt_r, pc)
    ppred = np.einsum("bnij,bnj->bni", r, pm) + pred_trans.astype(np.float64)
    tgt = points.astype(np.float64)
    if int(cls_id[0]) in SYM_CLASS_IDS:
        aa = np.sum(ppred * ppred, axis=-1)
        bb2 = np.sum(tgt * tgt, axis=-1)
        ab = np.einsum("bnd,bmd->bnm", ppred, tgt)
        d2 = aa[:, :, None] + bb2[:, None, :] - 2.0 * ab
        loss_dist = np.sqrt(np.maximum(d2, 1e-12)).min(axis=2)
    else:
        loss_dist = np.linalg.norm(ppred - tgt, axis=2)
    c = np.clip(pred_conf[..., 0].astype(np.float64), 1e-4, 1.0)
    return np.float32(np.mean(loss_dist * c - W_RATE * np.log(c)))


def kernel(pred_quat, pred_trans, pred_conf, pose, points, cls_id):
    pred_quat = _np_f32(pred_quat)
    pred_trans = _np_f32(pred_trans)
    pred_conf = _np_f32(pred_conf)
    pose = _np_f32(pose)
    points = _np_f32(points)
    cls_id = np.asarray(cls_id)

    assert pred_quat.shape == (B, N, 4), pred_quat.shape

    if int(cls_id[0]) not in SYM_CLASS_IDS:
        return np.array(
            _numpy_reference(pred_quat, pred_trans, pred_conf, pose, points,
                             cls_id),
            dtype=np.float32)

    from concourse.bass_utils import run_bass_kernel_spmd

    nc = _get_nc()
    in_maps = [
        {
            "pred_quat": np.ascontiguousarray(pred_quat[c]),
            "pred_trans": np.ascontiguousarray(pred_trans[c]),
            "pred_conf": np.ascontiguousarray(pred_conf[c]),
            "pose": np.ascontiguousarray(pose[c]),
            "points": np.ascontiguousarray(points[c]),
        }
        for c in range(B)
    ]
    res = run_bass_kernel_spmd(nc, in_maps, core_ids=list(range(B)))
    total = np.float64(0.0)
    for r in res.results:
        total += np.sum(r["out_sums"].astype(np.float64))
    return np.array(total / (B * N), dtype=np.float32)


# revision 17
# speedup vs baseline: 3.1414x; 1.2259x over previous
"""ADD-S (symmetric) pose loss kernel for Trainium2, 8 NeuronCores.

Sharding: data-parallel over the batch dim B=8 -> one batch element per core.
Each core computes sum_n [ min_dist(n) * conf(n) - W*log(conf(n)) ] for its
4096 points, returned as [128,1] per-partition partial sums; the host sums the
8*128 partials and divides by B*N.

Device algorithm (per core, N = 4096 points):
  1. Elementwise prologue on DVE in a SoA layout ([128 partitions, 32 free],
     point n lives at (p, f) = (n >> 5, n & 31)):
       - quat -> rotation via the unnormalized form R = M / |q|^2
       - points_model = R_gt @ (points - t_gt)      (per-batch scalars
         broadcast to [128,1] columns, applied with scalar_tensor_tensor)
       - points_pred  = R_pred @ points_model + trans
       - aa = |points_pred|^2, bb = |points|^2
  2. Quantize coordinates to bf16 (pp~, q~) and compute aa=|pp~|^2,
     bb=|q~|^2 in f32 FROM the quantized coords, carried as exact bf16
     hi+lo row pairs.  d2 = |pp~ - q~|^2 is then computed exactly for the
     perturbed points -- the bf16 error acts as a tiny zero-mean point
     perturbation instead of a catastrophic cancellation error.
     Operands (SBUF->SBUF flatten DMAs, identity order n = p*32 + f):
       lhsT [7, 4096] bf16 = [aa_h, aa_l, 1, 1, -2pp~_x, -2pp~_y, -2pp~_z]
       rhs  [7, 4096] bf16 = [1,    1, bb_h, bb_l,  q~_x,   q~_y,   q~_z]
  3. d2[n, m] as single-pass bf16 K=7 matmuls on the PE, packed 4x with
     row tiling: the 128x128 array runs in 32x128 mode, row-tile i holding
     the weights of n-block (i, g) at SBUF base partition 32i (rhs
     replicated at bases 0/32/64/96).  4 concurrent matmuls per issue.
     Block (i, g) covers the contiguous points n in [1024i+128g, +128).
  4. min over m split across two engines: 1-of-4 blocks reduced directly
     from PSUM f32 on DVE; the other 3 copied PSUM->SBUF as fp16 by the
     ScalarE (closest engine to PSUM) and min-folded on DVE at 2x rate
     (fp16 tensor_tensor tree), cutting the 16.7M-element scan cost.
  5. dist = sqrt(max(min_d2, 1e-12)); pixel = dist*clip(conf) - W*ln(clip(conf));
     per-partition row sums -> [128, 1] output.
"""

import numpy as np

B = 8
N = 4096
P = 128
F = N // P          # 32 free elems per partition in SoA layout
NB = N // P         # 32 n-blocks of 128
MSPAN = 2048        # PSUM reduce span (4 banks)
W_RATE = 0.015
SYM_CLASS_IDS = {1}

_cache = {}


def _np_f32(x):
    return np.ascontiguousarray(np.asarray(x), dtype=np.float32)


def _emit(ctx, tc, out_ap, ins):
    import concourse.bass as bass
    from concourse import mybir

    nc = tc.nc
    f32 = mybir.dt.float32
    Alu = mybir.AluOpType
    Act = mybir.ActivationFunctionType
    X = mybir.AxisListType.X

    quat, trans, conf, pose, points = (
        ins["pred_quat"], ins["pred_trans"], ins["pred_conf"],
        ins["pose"], ins["points"],
    )

    pool = ctx.enter_context(tc.tile_pool(name="main", bufs=1))

    def t(tag, shape, dtype=f32):
        return pool.tile(shape, dtype, tag=tag, name=tag)

    dma = nc.sync.dma_start

    # ---------------- input loads ----------------
    q_t = t("q_t", [P, F * 4])       # quat rows, 4 per point
    p_t = t("p_t", [P, F * 3])       # points
    tr_t = t("tr_t", [P, F * 3])     # pred_trans
    bc = t("bc", [P, 12])            # pose scalars broadcast across partitions
    conf_b = t("conf_b", [P, NB])    # conf in output (SoA-B) order

    nc.gpsimd.dma_start(out=q_t, in_=quat.rearrange("(p f) c -> p (f c)", p=P))
    nc.gpsimd.dma_start(out=p_t, in_=points.rearrange("(p f) c -> p (f c)", p=P))
    nc.gpsimd.dma_start(out=tr_t, in_=trans.rearrange("(p f) c -> p (f c)", p=P))
    nc.gpsimd.dma_start(out=bc, in_=bass.AP(tensor=pose.tensor,
                                            offset=pose.offset,
                                            ap=[[0, P], [1, 12]]))
    # conf gather in block order: conf_b[p, beta] = conf[beta*128 + p].
    # (minsb columns are col = 8i + g with beta = 4g + i; the tail ops
    # apply that permutation via strided views.)
    nc.gpsimd.dma_start(out=conf_b, in_=bass.AP(tensor=conf.tensor,
                                                offset=conf.offset,
                                                ap=[[1, P], [P, NB]]))

    q3 = q_t.rearrange("p (f c) -> p f c", c=4)
    p3 = p_t.rearrange("p (f c) -> p f c", c=3)
    tr3 = tr_t.rearrange("p (f c) -> p f c", c=3)

    vec = nc.vector
    from concourse.tile import add_dep_helper

    # DMA-wait funnel: a chain of TT ops absorbs every input-DMA semaphore
    # wait (1 per instruction) so downstream TensorScalar ops, which have
    # very few HW sync-wait slots, never carry DMA waits themselves.  All
    # early DVE consumers of DMA'd tiles are order-pinned after the funnel.
    scrf = t("scrf", [P, 1])
    vec.tensor_copy(out=scrf, in_=q_t[:, 0:1])
    for dep_t in (p_t, tr_t, bc, conf_b):
        last_f = vec.tensor_tensor(out=scrf, in0=scrf, in1=dep_t[:, 0:1],
                                   op=Alu.add)

    def pin(inst):
        add_dep_helper(inst.ins, last_f.ins, sync=False,
                       reason="order after input-DMA funnel")
        return inst

    # ---------------- conf term (early: ACT Ln table load overlaps) -------
    cc = t("cc", [P, NB])
    pin(vec.tensor_scalar_max(cc, conf_b, 1e-4))
    vec.tensor_scalar_min(cc, cc, 1.0)
    lnc = t("lnc", [P, NB])
    ln_inst = nc.scalar.activation(lnc, cc, Act.Ln)
    # prefetch the sqrt table set during the main loop (after all Ln uses)
    sq_pre = t("sq_pre", [P, 1])
    sq_inst = nc.scalar.sqrt(sq_pre, cc[:, 0:1])
    add_dep_helper(sq_inst.ins, ln_inst.ins, sync=False,
                   reason="load sqrt ACT table after ln")

    # ---------------- quaternion -> unnormalized rotation ----------------
    sq = t("sq", [P, F * 4])
    pin(vec.tensor_tensor(out=sq, in0=q_t, in1=q_t, op=Alu.mult))
    sq3 = sq.rearrange("p (f c) -> p f c", c=4)
    s2 = t("s2", [P, F])
    vec.reduce_sum(s2, sq3, axis=X)
    s2c = t("s2c", [P, F])
    vec.tensor_scalar_max(s2c, s2, 1e-16)
    rec = t("rec", [P, F])
    vec.reciprocal(rec, s2c)

    # gt transform: pm_k = sum_j Rg[k,j] * (points_j - t_j)
    # pose flat layout: Rg[k][j] = bc[:, 4k+j], t[j] = bc[:, 4j+3]
    # (per-partition-scalar ops lower to TensorScalarPtr: DVE-only)
    gp = nc.gpsimd
    pc = [t(f"pc{j}", [P, F]) for j in range(3)]
    for j in range(3):
        pin(vec.tensor_scalar_sub(pc[j], p3[:, :, j],
                                  bc[:, 4 * j + 3: 4 * j + 4]))
    pm = [t(f"pm{k}", [P, F]) for k in range(3)]
    for k in range(3):
        pin(vec.tensor_scalar_mul(pm[k], pc[0], bc[:, 4 * k: 4 * k + 1]))
        vec.scalar_tensor_tensor(out=pm[k], in0=pc[1],
                                 scalar=bc[:, 4 * k + 1: 4 * k + 2],
                                 in1=pm[k], op0=Alu.mult, op1=Alu.add)
        vec.scalar_tensor_tensor(out=pm[k], in0=pc[2],
                                 scalar=bc[:, 4 * k + 2: 4 * k + 3],
                                 in1=pm[k], op0=Alu.mult, op1=Alu.add)

    qw, qx, qy, qz = (q3[:, :, 0], q3[:, :, 1], q3[:, :, 2], q3[:, :, 3])
    xx, yy, zz = (sq3[:, :, 1], sq3[:, :, 2], sq3[:, :, 3])

    def dbl_prod(tag, a, b_):
        o = t(tag, [P, F])
        pin(vec.scalar_tensor_tensor(out=o, in0=a, scalar=2.0, in1=b_,
                                     op0=Alu.mult, op1=Alu.mult))
        return o

    xy2 = dbl_prod("xy2", qx, qy)
    xz2 = dbl_prod("xz2", qx, qz)
    yz2 = dbl_prod("yz2", qy, qz)
    wx2 = dbl_prod("wx2", qw, qx)
    wy2 = dbl_prod("wy2", qw, qy)
    wz2 = dbl_prod("wz2", qw, qz)

    def tt(tag, a, b_, op):
        o = t(tag, [P, F])
        vec.tensor_tensor(out=o, in0=a, in1=b_, op=op)
        return o

    b01m = tt("b01m", xy2, wz2, Alu.subtract)   # M[0][1]
    b01p = tt("b01p", xy2, wz2, Alu.add)        # M[1][0]
    b02p = tt("b02p", xz2, wy2, Alu.add)        # M[0][2]
    b02m = tt("b02m", xz2, wy2, Alu.subtract)   # M[2][0]
    b12m = tt("b12m", yz2, wx2, Alu.subtract)   # M[1][2]
    b12p = tt("b12p", yz2, wx2, Alu.add)        # M[2][1]

    a0 = tt("a0", yy, zz, Alu.add)
    a1 = tt("a1", xx, zz, Alu.add)
    a2 = tt("a2", xx, yy, Alu.add)
    u = []
    for k, ak in enumerate((a0, a1, a2)):
        uk = t(f"u{k}", [P, F])
        vec.scalar_tensor_tensor(out=uk, in0=ak, scalar=-2.0, in1=s2,
                                 op0=Alu.mult, op1=Alu.add)
        u.append(uk)

    # M rows (unnormalized R * s2):
    #   row0: [u0,   b01m, b02p]
    #   row1: [b01p, u1,   b12m]
    #   row2: [b02m, b12p, u2 ]
    rows = [(u[0], b01m, b02p), (b01p, u[1], b12m), (b02m, b12p, u[2])]
    pp = []
    scr = t("scr", [P, F])
    for i, (m0, m1, m2) in enumerate(rows):
        v = t(f"v{i}", [P, F])
        vec.tensor_tensor(out=v, in0=m0, in1=pm[0], op=Alu.mult)
        vec.tensor_tensor(out=scr, in0=m1, in1=pm[1], op=Alu.mult)
        vec.tensor_tensor(out=v, in0=v, in1=scr, op=Alu.add)
        vec.tensor_tensor(out=scr, in0=m2, in1=pm[2], op=Alu.mult)
        vec.tensor_tensor(out=v, in0=v, in1=scr, op=Alu.add)
        # pp_i = v * rec + trans_i
        vec.tensor_tensor(out=v, in0=v, in1=rec, op=Alu.mult)
        vec.tensor_tensor(out=v, in0=v, in1=tr3[:, :, i], op=Alu.add)
        pp.append(v)

    bf16 = mybir.dt.bfloat16

    # quantize predicted points to bf16; aa computed in f32 FROM the
    # quantized coords, then split into an exact bf16 hi+lo pair
    ppq, n2 = [], []
    for i in range(3):
        q_ = t(f"ppq{i}", [P, F], bf16)
        vec.tensor_copy(out=q_, in_=pp[i])
        ppq.append(q_)
        o = t(f"n2_{i}", [P, F], bf16)
        vec.tensor_scalar_mul(o, q_, -2.0)     # exact: *2 and bf16 input
        n2.append(o)
    aa = t("aa", [P, F])
    vec.tensor_tensor(out=aa, in0=ppq[0], in1=ppq[0], op=Alu.mult)
    vec.tensor_tensor(out=scr, in0=ppq[1], in1=ppq[1], op=Alu.mult)
    vec.tensor_tensor(out=aa, in0=aa, in1=scr, op=Alu.add)
    vec.tensor_tensor(out=scr, in0=ppq[2], in1=ppq[2], op=Alu.mult)
    vec.tensor_tensor(out=aa, in0=aa, in1=scr, op=Alu.add)
    aa_h = t("aa_h", [P, F], bf16)
    vec.tensor_copy(out=aa_h, in_=aa)
    aa_l = t("aa_l", [P, F], bf16)
    vec.tensor_tensor(out=aa_l, in0=aa, in1=aa_h, op=Alu.subtract)

    # quantized target coords + bb = |q~|^2 as exact hi+lo pair (GpSimd)
    gscr = t("gscr", [P, F])
    pcomp = []
    for j in range(3):
        o = t(f"pcomp{j}", [P, F], bf16)
        gp.tensor_copy(out=o, in_=p3[:, :, j])
        pcomp.append(o)
    bb = t("bb", [P, F])
    gp.tensor_tensor(out=bb, in0=pcomp[0], in1=pcomp[0], op=Alu.mult)
    gp.tensor_tensor(out=gscr, in0=pcomp[1], in1=pcomp[1], op=Alu.mult)
    gp.tensor_tensor(out=bb, in0=bb, in1=gscr, op=Alu.add)
    gp.tensor_tensor(out=gscr, in0=pcomp[2], in1=pcomp[2], op=Alu.mult)
    gp.tensor_tensor(out=bb, in0=bb, in1=gscr, op=Alu.add)
    bb_h = t("bb_h", [P, F], bf16)
    gp.tensor_copy(out=bb_h, in_=bb)
    bb_l = t("bb_l", [P, F], bf16)
    gp.tensor_tensor(out=bb_l, in0=bb, in1=bb_h, op=Alu.subtract)


    # ---------------- matmul operands ----------------
    # lhsT packed for 4x row tiling: row-tile i (base partition 32i) holds
    # blocks (i, g) at columns [g*128, (g+1)*128).  The flatten DMA for
    # logical row r writes partitions {r, 32+r, 64+r, 96+r} in one shot:
    # dst partition-dim stride 32, and the source [128, 32] tile streams
    # its partitions in exactly the right order (n = p*32 + f).
    K_DIM = 7
    lhsT = t("lhsT", [P, N], bf16)
    rhs = t("rhs", [P, N], bf16)
    # Both operands: logical rows 0..6 at partitions 0..6 (contiguous,
    # identity flatten n = p*32 + f), then replicated to partition bases
    # 32/64/96 so each PE row tile streams from its own partitions.
    # lhsT row DMAs issue on the Scalar engine's HW-DGE, rhs rows on
    # Sync, so the two operand builds overlap.
    adma = nc.scalar.dma_start
    ones_t = t("ones_t", [P, F], bf16)
    vec.memset(ones_t, 1.0)
    adma(out=lhsT[2:3, :], in_=ones_t)
    adma(out=lhsT[3:4, :], in_=ones_t)
    dma(out=rhs[0:1, :], in_=ones_t)
    dma(out=rhs[1:2, :], in_=ones_t)
    for r, row in ((0, aa_h), (1, aa_l), (4, n2[0]), (5, n2[1]), (6, n2[2])):
        adma(out=lhsT[r:r + 1, :], in_=row)
    for r, row in ((2, bb_h), (3, bb_l),
                   (4, pcomp[0]), (5, pcomp[1]), (6, pcomp[2])):
        dma(out=rhs[r:r + 1, :], in_=row)
    for base in (32, 64, 96):
        adma(out=lhsT[base:base + K_DIM, :], in_=lhsT[0:K_DIM, :])
        dma(out=rhs[base:base + K_DIM, :], in_=rhs[0:K_DIM, :])

    # ---------------- main loop: packed d2 matmuls + split min ----------
    fp16 = mybir.dt.float16
    CH = 1024                       # psum chunk (2 banks); 4 chunks per block
    minsd = t("minsd", [P, 8 * 4])  # direct-path chunk mins, cols g*4 + c
    minsb = t("minsb", [P, NB])     # per-block mins, col = 8i + g
    pp_psum = ctx.enter_context(tc.tile_pool(name="d2p", bufs=4, space="PSUM"))
    fpool = ctx.enter_context(tc.tile_pool(name="f16", bufs=8))
    tpool = ctx.enter_context(tc.tile_pool(name="tree", bufs=2))
    for g in range(8):
        fchunks = {i: [] for i in (1, 2, 3)}
        for c in range(4):
            pts = [pp_psum.tile([P, CH], f32, tag="ps", name=f"ps{g}_{c}_{i}")
                   for i in range(4)]
            for j in range(2):
                for i in range(4):
                    beta = 4 * g + i          # block id; covers n in
                    nc.tensor.matmul(         # [beta*128, (beta+1)*128)
                        pts[i][:, j * 512:(j + 1) * 512],
                        lhsT[32 * i: 32 * i + K_DIM,
                             beta * P:(beta + 1) * P],
                        rhs[32 * i: 32 * i + K_DIM,
                            c * CH + j * 512: c * CH + (j + 1) * 512],
                        start=True, stop=True,
                        tile_position=(32 * i, 0),
                    )
            # consumer split: row-tile 0 -> direct f32 reduce on DVE;
            # row-tiles 1-3 -> ScalarE copy to fp16 SBUF
            col = g * 4 + c
            vec.tensor_reduce(minsd[:, col:col + 1], pts[0], axis=X,
                              op=Alu.min)
            for i in (1, 2, 3):
                fc = fpool.tile([P, CH], fp16, tag=f"fc{i}",
                                name=f"fc{g}_{c}_{i}")
                nc.scalar.copy(out=fc, in_=pts[i])
                fchunks[i].append(fc)
        # fp16 min trees for this g's row-tiles 1-3 (DVE, 2x rate)
        for i in (1, 2, 3):
            c0, c1, c2, c3 = fchunks[i]
            ta = tpool.tile([P, CH], fp16, tag="ta", name=f"ta{g}_{i}")
            tb = tpool.tile([P, CH], fp16, tag="tb", name=f"tb{g}_{i}")
            vec.tensor_tensor(out=ta, in0=c0, in1=c1, op=Alu.min)
            vec.tensor_tensor(out=tb, in0=c2, in1=c3, op=Alu.min)
            vec.tensor_tensor(out=ta, in0=ta, in1=tb, op=Alu.min)
            vec.tensor_tensor(out=ta[:, 0:512], in0=ta[:, 0:512],
                              in1=ta[:, 512:1024], op=Alu.min)
            vec.tensor_tensor(out=ta[:, 0:256], in0=ta[:, 0:256],
                              in1=ta[:, 256:512], op=Alu.min)
            vec.tensor_tensor(out=ta[:, 0:128], in0=ta[:, 0:128],
                              in1=ta[:, 128:256], op=Alu.min)
            col = 8 * i + g
            vec.tensor_reduce(minsb[:, col:col + 1], ta[:, 0:128], axis=X,
                              op=Alu.min)

    # direct-path blocks (i=0): fold the 4 chunk mins -> cols g of minsb
    vec.tensor_reduce(minsb[:, 0:8],
                      minsd.rearrange("p (g c) -> p g c", c=4), axis=X,
                      op=Alu.min)

    # ---------------- tail: sqrt, pixel loss, row sums ----------
    vec.tensor_scalar_max(minsb, minsb, 1e-12)
    dist = t("dist", [P, NB])
    nc.scalar.sqrt(dist, minsb)

    # dist columns are 8i+g; cc/lnc columns are beta = 4g+i -> use
    # permuted free-dim views of cc/lnc to line the blocks up.
    pix = t("pix", [P, NB])
    pix_v = pix.rearrange("p (i g) -> p i g", g=8)
    cc_v = cc.rearrange("p (g i) -> p i g", i=4)
    lnc_v = lnc.rearrange("p (g i) -> p i g", i=4)
    dist_v = dist.rearrange("p (i g) -> p i g", g=8)
    vec.tensor_tensor(out=pix_v, in0=dist_v, in1=cc_v, op=Alu.mult)
    vec.scalar_tensor_tensor(out=pix_v, in0=lnc_v, scalar=-W_RATE, in1=pix_v,
                             op0=Alu.mult, op1=Alu.add)
    sums = t("sums", [P, 1])
    vec.reduce_sum(sums, pix, axis=X)
    dma(out=out_ap, in_=sums)


def _build():
    from contextlib import ExitStack

    import concourse.bacc as bacc
    import concourse.tile as tile
    from concourse import mybir

    f32 = mybir.dt.float32
    nc = bacc.Bacc("TRN2", debug=False, enable_asserts=False, num_devices=B)
    ins = {
        "pred_quat": nc.dram_tensor("pred_quat", [N, 4], f32,
                                    kind="ExternalInput").ap(),
        "pred_trans": nc.dram_tensor("pred_trans", [N, 3], f32,
                                     kind="ExternalInput").ap(),
        "pred_conf": nc.dram_tensor("pred_conf", [N, 1], f32,
                                    kind="ExternalInput").ap(),
        "pose": nc.dram_tensor("pose", [3, 4], f32, kind="ExternalInput").ap(),
        "points": nc.dram_tensor("points", [N, 3], f32,
                                 kind="ExternalInput").ap(),
    }
    out_ap = nc.dram_tensor("out_sums", [P, 1], f32, kind="ExternalOutput").ap()
    with tile.TileContext(nc) as tc:
        with ExitStack() as ctx:
            _emit(ctx, tc, out_ap, ins)
    nc.compile()
    return nc


def _get_nc():
    if "nc" not in _cache:
        _cache["nc"] = _build()
    return _cache["nc"]


def _numpy_reference(pred_quat, pred_trans, pred_conf, pose, points, cls_id):
    """Full-precision numpy fallback (used only for the non-symmetric branch)."""
    q = pred_quat.astype(np.float64)
    q = q / np.clip(np.linalg.norm(q, axis=-1, keepdims=True), 1e-8, None)
    w, x, y, z = q[..., 0], q[..., 1], q[..., 2], q[..., 3]
    r = np.stack([
        1 - 2 * (y * y + z * z), 2 * (x * y - w * z), 2 * (x * z + w * y),
        2 * (x * y + w * z), 1 - 2 * (x * x + z * z), 2 * (y * z - w * x),
        2 * (x * z - w * y), 2 * (y * z + w * x), 1 - 2 * (x * x + y * y),
    ], axis=-1).reshape(q.shape[:-1] + (3, 3))
    gt_r = pose[:, :3, :3].astype(np.float64)
    gt_t = pose[:, :3, 3].astype(np.float64)
    pc = points.astype(np.float64) - gt_t[:, None, :]
    pm = np.einsum("bkj,bnj->bnk", gt_r, pc)
    ppred = np.einsum("bnij,bnj->bni", r, pm) + pred_trans.astype(np.float64)
    tgt = points.astype(np.float64)
    if int(cls_id[0]) in SYM_CLASS_IDS:
        aa = np.sum(ppred * ppred, axis=-1)
        bb2 = np.sum(tgt * tgt, axis=-1)
        ab = np.einsum("bnd,bmd->bnm", ppred, tgt)
        d2 = aa[:, :, None] + bb2[:, None, :] - 2.0 * ab
        loss_dist = np.sqrt(np.maximum(d2, 1e-12)).min(axis=2)
    else:
        loss_dist = np.linalg.norm(ppred - tgt, axis=2)
    c = np.clip(pred_conf[..., 0].astype(np.float64), 1e-4, 1.0)
    return np.float32(np.mean(loss_dist * c - W_RATE * np.log(c)))


def kernel(pred_quat, pred_trans, pred_conf, pose, points, cls_id):
    pred_quat = _np_f32(pred_quat)
    pred_trans = _np_f32(pred_trans)
    pred_conf = _np_f32(pred_conf)
    pose = _np_f32(pose)
    points = _np_f32(points)
    cls_id = np.asarray(cls_id)

    assert pred_quat.shape == (B, N, 4), pred_quat.shape

    if int(cls_id[0]) not in SYM_CLASS_IDS:
        return np.array(
            _numpy_reference(pred_quat, pred_trans, pred_conf, pose, points,
                             cls_id),
            dtype=np.float32)

    from concourse.bass_utils import run_bass_kernel_spmd

    nc = _get_nc()
    in_maps = [
        {
            "pred_quat": np.ascontiguousarray(pred_quat[c]),
            "pred_trans": np.ascontiguousarray(pred_trans[c]),
            "pred_conf": np.ascontiguousarray(pred_conf[c]),
            "pose": np.ascontiguousarray(pose[c]),
            "points": np.ascontiguousarray(points[c]),
        }
        for c in range(B)
    ]
    res = run_bass_kernel_spmd(nc, in_maps, core_ids=list(range(B)))
    total = np.float64(0.0)
    for r in res.results:
        total += np.sum(r["out_sums"].astype(np.float64))
    return np.array(total / (B * N), dtype=np.float32)


# revision 19
# speedup vs baseline: 3.1835x; 1.0134x over previous
"""ADD-S (symmetric) pose loss kernel for Trainium2, 8 NeuronCores.

Sharding: data-parallel over the batch dim B=8 -> one batch element per core.
Each core computes sum_n [ min_dist(n) * conf(n) - W*log(conf(n)) ] for its
4096 points, returned as [128,1] per-partition partial sums; the host sums the
8*128 partials and divides by B*N.

Device algorithm (per core, N = 4096 points):
  1. Elementwise prologue on DVE in a SoA layout ([128 partitions, 32 free],
     point n lives at (p, f) = (n >> 5, n & 31)):
       - quat -> rotation via the unnormalized form R = M / |q|^2
       - points_model = R_gt @ (points - t_gt)      (per-batch scalars
         broadcast to [128,1] columns, applied with scalar_tensor_tensor)
       - points_pred  = R_pred @ points_model + trans
       - aa = |points_pred|^2, bb = |points|^2
  2. Quantize coordinates to bf16 (pp~, q~) and compute aa=|pp~|^2,
     bb=|q~|^2 in f32 FROM the quantized coords, carried as exact bf16
     hi+lo row pairs.  d2 = |pp~ - q~|^2 is then computed exactly for the
     perturbed points -- the bf16 error acts as a tiny zero-mean point
     perturbation instead of a catastrophic cancellation error.
     Operands (SBUF->SBUF flatten DMAs, identity order n = p*32 + f):
       lhsT [7, 4096] bf16 = [aa_h, aa_l, 1, 1, -2pp~_x, -2pp~_y, -2pp~_z]
       rhs  [7, 4096] bf16 = [1,    1, bb_h, bb_l,  q~_x,   q~_y,   q~_z]
  3. d2[n, m] as single-pass bf16 K=7 matmuls on the PE, packed 4x with
     row tiling: the 128x128 array runs in 32x128 mode, row-tile i holding
     the weights of n-block (i, g) at SBUF base partition 32i (rhs
     replicated at bases 0/32/64/96).  4 concurrent matmuls per issue.
     Block (i, g) covers the contiguous points n in [1024i+128g, +128).
  4. min over m split across two engines: 1-of-4 blocks reduced directly
     from PSUM f32 on DVE; the other 3 copied PSUM->SBUF as fp16 by the
     ScalarE (closest engine to PSUM) and min-folded on DVE at 2x rate
     (fp16 tensor_tensor tree), cutting the 16.7M-element scan cost.
  5. dist = sqrt(max(min_d2, 1e-12)); pixel = dist*clip(conf) - W*ln(clip(conf));
     per-partition row sums -> [128, 1] output.
"""

import numpy as np

B = 8
N = 4096
P = 128
F = N // P          # 32 free elems per partition in SoA layout
NB = N // P         # 32 n-blocks of 128
MSPAN = 2048        # PSUM reduce span (4 banks)
W_RATE = 0.015
SYM_CLASS_IDS = {1}

_cache = {}


def _np_f32(x):
    return np.ascontiguousarray(np.asarray(x), dtype=np.float32)


def _emit(ctx, tc, out_ap, ins):
    import concourse.bass as bass
    from concourse import mybir

    nc = tc.nc
    f32 = mybir.dt.float32
    Alu = mybir.AluOpType
    Act = mybir.ActivationFunctionType
    X = mybir.AxisListType.X

    quat, trans, conf, pose, points = (
        ins["pred_quat"], ins["pred_trans"], ins["pred_conf"],
        ins["pose"], ins["points"],
    )

    pool = ctx.enter_context(tc.tile_pool(name="main", bufs=1))

    def t(tag, shape, dtype=f32):
        return pool.tile(shape, dtype, tag=tag, name=tag)

    dma = nc.sync.dma_start

    # ---------------- input loads ----------------
    q_t = t("q_t", [P, F * 4])       # quat rows, 4 per point
    p_t = t("p_t", [P, F * 3])       # points
    tr_t = t("tr_t", [P, F * 3])     # pred_trans
    bc = t("bc", [P, 12])            # pose scalars broadcast across partitions
    conf_b = t("conf_b", [P, NB])    # conf in output (SoA-B) order

    nc.gpsimd.dma_start(out=q_t, in_=quat.rearrange("(p f) c -> p (f c)", p=P))
    nc.gpsimd.dma_start(out=p_t, in_=points.rearrange("(p f) c -> p (f c)", p=P))
    nc.gpsimd.dma_start(out=tr_t, in_=trans.rearrange("(p f) c -> p (f c)", p=P))
    nc.gpsimd.dma_start(out=bc, in_=bass.AP(tensor=pose.tensor,
                                            offset=pose.offset,
                                            ap=[[0, P], [1, 12]]))

    q3 = q_t.rearrange("p (f c) -> p f c", c=4)
    p3 = p_t.rearrange("p (f c) -> p f c", c=3)
    tr3 = tr_t.rearrange("p (f c) -> p f c", c=3)

    vec = nc.vector
    from concourse.tile import add_dep_helper

    # DMA-wait funnel: a chain of TT ops absorbs every input-DMA semaphore
    # wait (1 per instruction) so downstream TensorScalar ops, which have
    # very few HW sync-wait slots, never carry DMA waits themselves.  All
    # early DVE consumers of DMA'd tiles are order-pinned after the funnel.
    scrf = t("scrf", [P, 1])
    vec.tensor_copy(out=scrf, in_=q_t[:, 0:1])
    for dep_t in (p_t, tr_t, bc):
        last_f = vec.tensor_tensor(out=scrf, in0=scrf, in1=dep_t[:, 0:1],
                                   op=Alu.add)

    def pin(inst):
        add_dep_helper(inst.ins, last_f.ins, sync=False,
                       reason="order after input-DMA funnel")
        return inst

    # ---------------- quaternion -> unnormalized rotation ----------------
    sq = t("sq", [P, F * 4])
    pin(vec.tensor_tensor(out=sq, in0=q_t, in1=q_t, op=Alu.mult))
    sq3 = sq.rearrange("p (f c) -> p f c", c=4)
    s2 = t("s2", [P, F])
    vec.reduce_sum(s2, sq3, axis=X)
    s2c = t("s2c", [P, F])
    vec.tensor_scalar_max(s2c, s2, 1e-16)
    rec = t("rec", [P, F])
    vec.reciprocal(rec, s2c)

    # gt transform: pm_k = sum_j Rg[k,j] * (points_j - t_j)
    # pose flat layout: Rg[k][j] = bc[:, 4k+j], t[j] = bc[:, 4j+3]
    # (per-partition-scalar ops lower to TensorScalarPtr: DVE-only)
    gp = nc.gpsimd
    pc = [t(f"pc{j}", [P, F]) for j in range(3)]
    for j in range(3):
        pin(vec.tensor_scalar_sub(pc[j], p3[:, :, j],
                                  bc[:, 4 * j + 3: 4 * j + 4]))
    pm = [t(f"pm{k}", [P, F]) for k in range(3)]
    for k in range(3):
        pin(vec.tensor_scalar_mul(pm[k], pc[0], bc[:, 4 * k: 4 * k + 1]))
        vec.scalar_tensor_tensor(out=pm[k], in0=pc[1],
                                 scalar=bc[:, 4 * k + 1: 4 * k + 2],
                                 in1=pm[k], op0=Alu.mult, op1=Alu.add)
        vec.scalar_tensor_tensor(out=pm[k], in0=pc[2],
                                 scalar=bc[:, 4 * k + 2: 4 * k + 3],
                                 in1=pm[k], op0=Alu.mult, op1=Alu.add)

    qw, qx, qy, qz = (q3[:, :, 0], q3[:, :, 1], q3[:, :, 2], q3[:, :, 3])
    xx, yy, zz = (sq3[:, :, 1], sq3[:, :, 2], sq3[:, :, 3])

    def dbl_prod(tag, a, b_):
        o = t(tag, [P, F])
        pin(vec.scalar_tensor_tensor(out=o, in0=a, scalar=2.0, in1=b_,
                                     op0=Alu.mult, op1=Alu.mult))
        return o

    xy2 = dbl_prod("xy2", qx, qy)
    xz2 = dbl_prod("xz2", qx, qz)
    yz2 = dbl_prod("yz2", qy, qz)
    wx2 = dbl_prod("wx2", qw, qx)
    wy2 = dbl_prod("wy2", qw, qy)
    wz2 = dbl_prod("wz2", qw, qz)

    def tt(tag, a, b_, op):
        o = t(tag, [P, F])
        vec.tensor_tensor(out=o, in0=a, in1=b_, op=op)
        return o

    b01m = tt("b01m", xy2, wz2, Alu.subtract)   # M[0][1]
    b01p = tt("b01p", xy2, wz2, Alu.add)        # M[1][0]
    b02p = tt("b02p", xz2, wy2, Alu.add)        # M[0][2]
    b02m = tt("b02m", xz2, wy2, Alu.subtract)   # M[2][0]
    b12m = tt("b12m", yz2, wx2, Alu.subtract)   # M[1][2]
    b12p = tt("b12p", yz2, wx2, Alu.add)        # M[2][1]

    a0 = tt("a0", yy, zz, Alu.add)
    a1 = tt("a1", xx, zz, Alu.add)
    a2 = tt("a2", xx, yy, Alu.add)
    u = []
    for k, ak in enumerate((a0, a1, a2)):
        uk = t(f"u{k}", [P, F])
        vec.scalar_tensor_tensor(out=uk, in0=ak, scalar=-2.0, in1=s2,
                                 op0=Alu.mult, op1=Alu.add)
        u.append(uk)

    # M rows (unnormalized R * s2):
    #   row0: [u0,   b01m, b02p]
    #   row1: [b01p, u1,   b12m]
    #   row2: [b02m, b12p, u2 ]
    rows = [(u[0], b01m, b02p), (b01p, u[1], b12m), (b02m, b12p, u[2])]
    pp = []
    scr = t("scr", [P, F])
    for i, (m0, m1, m2) in enumerate(rows):
        v = t(f"v{i}", [P, F])
        vec.tensor_tensor(out=v, in0=m0, in1=pm[0], op=Alu.mult)
        vec.tensor_tensor(out=scr, in0=m1, in1=pm[1], op=Alu.mult)
        vec.tensor_tensor(out=v, in0=v, in1=scr, op=Alu.add)
        vec.tensor_tensor(out=scr, in0=m2, in1=pm[2], op=Alu.mult)
        vec.tensor_tensor(out=v, in0=v, in1=scr, op=Alu.add)
        # pp_i = v * rec + trans_i
        vec.tensor_tensor(out=v, in0=v, in1=rec, op=Alu.mult)
        vec.tensor_tensor(out=v, in0=v, in1=tr3[:, :, i], op=Alu.add)
        pp.append(v)

    bf16 = mybir.dt.bfloat16

    # quantize predicted points to bf16; aa computed in f32 FROM the
    # quantized coords, then split into an exact bf16 hi+lo pair
    ppq, n2 = [], []
    for i in range(3):
        q_ = t(f"ppq{i}", [P, F], bf16)
        vec.tensor_copy(out=q_, in_=pp[i])
        ppq.append(q_)
        o = t(f"n2_{i}", [P, F], bf16)
        vec.tensor_scalar_mul(o, q_, -2.0)     # exact: *2 and bf16 input
        n2.append(o)
    aa = t("aa", [P, F])
    vec.tensor_tensor(out=aa, in0=ppq[0], in1=ppq[0], op=Alu.mult)
    vec.tensor_tensor(out=scr, in0=ppq[1], in1=ppq[1], op=Alu.mult)
    vec.tensor_tensor(out=aa, in0=aa, in1=scr, op=Alu.add)
    vec.tensor_tensor(out=scr, in0=ppq[2], in1=ppq[2], op=Alu.mult)
    vec.tensor_tensor(out=aa, in0=aa, in1=scr, op=Alu.add)
    aa_h = t("aa_h", [P, F], bf16)
    vec.tensor_copy(out=aa_h, in_=aa)
    aa_l = t("aa_l", [P, F], bf16)
    vec.tensor_tensor(out=aa_l, in0=aa, in1=aa_h, op=Alu.subtract)

    # quantized target coords + bb = |q~|^2 as exact hi+lo pair (GpSimd)
    gscr = t("gscr", [P, F])
    pcomp = []
    for j in range(3):
        o = t(f"pcomp{j}", [P, F], bf16)
        gp.tensor_copy(out=o, in_=p3[:, :, j])
        pcomp.append(o)
    bb = t("bb", [P, F])
    gp.tensor_tensor(out=bb, in0=pcomp[0], in1=pcomp[0], op=Alu.mult)
    gp.tensor_tensor(out=gscr, in0=pcomp[1], in1=pcomp[1], op=Alu.mult)
    gp.tensor_tensor(out=bb, in0=bb, in1=gscr, op=Alu.add)
    gp.tensor_tensor(out=gscr, in0=pcomp[2], in1=pcomp[2], op=Alu.mult)
    gp.tensor_tensor(out=bb, in0=bb, in1=gscr, op=Alu.add)
    bb_h = t("bb_h", [P, F], bf16)
    gp.tensor_copy(out=bb_h, in_=bb)
    bb_l = t("bb_l", [P, F], bf16)
    gp.tensor_tensor(out=bb_l, in0=bb, in1=bb_h, op=Alu.subtract)

    # conf gather in block order: conf_b[p, beta] = conf[beta*128 + p];
    # emitted after the gpsimd compute chain (needed only mid-loop).
    gp.dma_start(out=conf_b, in_=bass.AP(tensor=conf.tensor,
                                         offset=conf.offset,
                                         ap=[[1, P], [P, NB]]))

    # ---------------- conf term (early: ACT Ln table load overlaps) -------
    cc = t("cc", [P, NB])
    pin(vec.tensor_scalar_max(cc, conf_b, 1e-4))
    vec.tensor_scalar_min(cc, cc, 1.0)
    lnc = t("lnc", [P, NB])
    ln_inst = nc.scalar.activation(lnc, cc, Act.Ln)
    # prefetch the sqrt table set during the main loop (after all Ln uses)
    sq_pre = t("sq_pre", [P, 1])
    sq_inst = nc.scalar.sqrt(sq_pre, cc[:, 0:1])
    add_dep_helper(sq_inst.ins, ln_inst.ins, sync=False,
                   reason="load sqrt ACT table after ln")



    # ---------------- matmul operands ----------------
    # lhsT packed for 4x row tiling: row-tile i (base partition 32i) holds
    # blocks (i, g) at columns [g*128, (g+1)*128).  The flatten DMA for
    # logical row r writes partitions {r, 32+r, 64+r, 96+r} in one shot:
    # dst partition-dim stride 32, and the source [128, 32] tile streams
    # its partitions in exactly the right order (n = p*32 + f).
    K_DIM = 7
    lhsT = t("lhsT", [P, N], bf16)
    rhs = t("rhs", [P, N], bf16)
    # Both operands: logical rows 0..6 at partitions 0..6 (contiguous,
    # identity flatten n = p*32 + f), then replicated to partition bases
    # 32/64/96 so each PE row tile streams from its own partitions.
    # lhsT row DMAs issue on the Scalar engine's HW-DGE, rhs rows on
    # Sync, so the two operand builds overlap.
    adma = nc.scalar.dma_start
    ones_t = t("ones_t", [P, F], bf16)
    vec.memset(ones_t, 1.0)
    adma(out=lhsT[2:3, :], in_=ones_t)
    adma(out=lhsT[3:4, :], in_=ones_t)
    dma(out=rhs[0:1, :], in_=ones_t)
    dma(out=rhs[1:2, :], in_=ones_t)
    for r, row in ((0, aa_h), (1, aa_l), (4, n2[0]), (5, n2[1]), (6, n2[2])):
        adma(out=lhsT[r:r + 1, :], in_=row)
    for r, row in ((2, bb_h), (3, bb_l),
                   (4, pcomp[0]), (5, pcomp[1]), (6, pcomp[2])):
        dma(out=rhs[r:r + 1, :], in_=row)
    for base in (32, 64, 96):
        adma(out=lhsT[base:base + K_DIM, :], in_=lhsT[0:K_DIM, :])
        dma(out=rhs[base:base + K_DIM, :], in_=rhs[0:K_DIM, :])

    # ---------------- main loop: packed d2 matmuls + split min ----------
    fp16 = mybir.dt.float16
    CH = 1024                       # psum chunk (2 banks); 4 chunks per block
    minsd = t("minsd", [P, 8 * 4])  # direct-path chunk mins, cols g*4 + c
    minsb = t("minsb", [P, NB])     # per-block mins, col = 8i + g
    pp_psum = ctx.enter_context(tc.tile_pool(name="d2p", bufs=4, space="PSUM"))
    fpool = ctx.enter_context(tc.tile_pool(name="f16", bufs=14))
    tpool = ctx.enter_context(tc.tile_pool(name="tree", bufs=2))
    for g in range(8):
        fchunks = {i: [] for i in (1, 2, 3)}
        for c in range(4):
            pts = [pp_psum.tile([P, CH], f32, tag="ps", name=f"ps{g}_{c}_{i}")
                   for i in range(4)]
            for j in range(2):
                for i in range(4):
                    beta = 4 * g + i          # block id; covers n in
                    nc.tensor.matmul(         # [beta*128, (beta+1)*128)
                        pts[i][:, j * 512:(j + 1) * 512],
                        lhsT[32 * i: 32 * i + K_DIM,
                             beta * P:(beta + 1) * P],
                        rhs[32 * i: 32 * i + K_DIM,
                            c * CH + j * 512: c * CH + (j + 1) * 512],
                        start=True, stop=True,
                        tile_position=(32 * i, 0),
                    )
            # consumer split: row-tile 0 -> direct f32 reduce on DVE;
            # row-tiles 1-3 -> ScalarE copy to fp16 SBUF
            col = g * 4 + c
            vec.tensor_reduce(minsd[:, col:col + 1], pts[0], axis=X,
                              op=Alu.min)
            for i in (1, 2, 3):
                fc = fpool.tile([P, CH], fp16, tag=f"fc{i}",
                                name=f"fc{g}_{c}_{i}")
                nc.scalar.copy(out=fc, in_=pts[i])
                fchunks[i].append(fc)
        # fp16 min trees for this g's row-tiles 1-3 (DVE, 2x rate)
        for i in (1, 2, 3):
            c0, c1, c2, c3 = fchunks[i]
            ta = tpool.tile([P, CH], fp16, tag="ta", name=f"ta{g}_{i}")
            tb = tpool.tile([P, CH], fp16, tag="tb", name=f"tb{g}_{i}")
            vec.tensor_tensor(out=ta, in0=c0, in1=c1, op=Alu.min)
            vec.tensor_tensor(out=tb, in0=c2, in1=c3, op=Alu.min)
            vec.tensor_tensor(out=ta, in0=ta, in1=tb, op=Alu.min)
            vec.tensor_tensor(out=ta[:, 0:512], in0=ta[:, 0:512],
                              in1=ta[:, 512:1024], op=Alu.min)
            vec.tensor_tensor(out=ta[:, 0:256], in0=ta[:, 0:256],
                              in1=ta[:, 256:512], op=Alu.min)
            vec.tensor_tensor(out=ta[:, 0:128], in0=ta[:, 0:128],
                              in1=ta[:, 128:256], op=Alu.min)
            col = 8 * i + g
            vec.tensor_reduce(minsb[:, col:col + 1], ta[:, 0:128], axis=X,
                              op=Alu.min)

    # direct-path blocks (i=0): fold the 4 chunk mins -> cols g of minsb
    vec.tensor_reduce(minsb[:, 0:8],
                      minsd.rearrange("p (g c) -> p g c", c=4), axis=X,
                      op=Alu.min)

    # ---------------- tail: sqrt, pixel loss, row sums ----------
    vec.tensor_scalar_max(minsb, minsb, 1e-12)
    dist = t("dist", [P, NB])
    nc.scalar.sqrt(dist, minsb)

    # dist columns are 8i+g; cc/lnc columns are beta = 4g+i -> use
    # permuted free-dim views of cc/lnc to line the blocks up.
    pix = t("pix", [P, NB])
    pix_v = pix.rearrange("p (i g) -> p i g", g=8)
    cc_v = cc.rearrange("p (g i) -> p i g", i=4)
    lnc_v = lnc.rearrange("p (g i) -> p i g", i=4)
    dist_v = dist.rearrange("p (i g) -> p i g", g=8)
    vec.tensor_tensor(out=pix_v, in0=dist_v, in1=cc_v, op=Alu.mult)
    vec.scalar_tensor_tensor(out=pix_v, in0=lnc_v, scalar=-W_RATE, in1=pix_v,
                             op0=Alu.mult, op1=Alu.add)
    sums = t("sums", [P, 1])
    vec.reduce_sum(sums, pix, axis=X)
    dma(out=out_ap, in_=sums)


def _build():
    from contextlib import ExitStack

    import concourse.bacc as bacc
    import concourse.tile as tile
    from concourse import mybir

    f32 = mybir.dt.float32
    nc = bacc.Bacc("TRN2", debug=False, enable_asserts=False, num_devices=B)
    ins = {
        "pred_quat": nc.dram_tensor("pred_quat", [N, 4], f32,
                                    kind="ExternalInput").ap(),
        "pred_trans": nc.dram_tensor("pred_trans", [N, 3], f32,
                                     kind="ExternalInput").ap(),
        "pred_conf": nc.dram_tensor("pred_conf", [N, 1], f32,
                                    kind="ExternalInput").ap(),
        "pose": nc.dram_tensor("pose", [3, 4], f32, kind="ExternalInput").ap(),
        "points": nc.dram_tensor("points", [N, 3], f32,
                                 kind="ExternalInput").ap(),
    }
    out_ap = nc.dram_tensor("out_sums", [P, 1], f32, kind="ExternalOutput").ap()
    with tile.TileContext(nc) as tc:
        with ExitStack() as ctx:
            _emit(ctx, tc, out_ap, ins)
    nc.compile()
    return nc


def _get_nc():
    if "nc" not in _cache:
        _cache["nc"] = _build()
    return _cache["nc"]


def _numpy_reference(pred_quat, pred_trans, pred_conf, pose, points, cls_id):
    """Full-precision numpy fallback (used only for the non-symmetric branch)."""
    q = pred_quat.astype(np.float64)
    q = q / np.clip(np.linalg.norm(q, axis=-1, keepdims=True), 1e-8, None)
    w, x, y, z = q[..., 0], q[..., 1], q[..., 2], q[..., 3]
    r = np.stack([
        1 - 2 * (y * y + z * z), 2 * (x * y - w * z), 2 * (x * z + w * y),
        2 * (x * y + w * z), 1 - 2 * (x * x + z * z), 2 * (y * z - w * x),
        2 * (x * z - w * y), 2 * (y * z + w * x), 1 - 2 * (x * x + y * y),
    ], axis=-1).reshape(q.shape[:-1] + (3, 3))
    gt_r = pose[:, :3, :3].astype(np.float64)
    gt_t = pose[:, :3, 3].astype(np.float64)
    pc = points.astype(np.float64) - gt_t[:, None, :]
    pm = np.einsum("bkj,bnj->bnk", gt_r, pc)
    ppred = np.einsum("bnij,bnj->bni", r, pm) + pred_trans.astype(np.float64)
    tgt = points.astype(np.float64)
    if int(cls_id[0]) in SYM_CLASS_IDS:
        aa = np.sum(ppred * ppred, axis=-1)
        bb2 = np.sum(tgt * tgt, axis=-1)
        ab = np.einsum("bnd,bmd->bnm", ppred, tgt)
        d2 = aa[:, :, None] + bb2[:, None, :] - 2.0 * ab
        loss_dist = np.sqrt(np.maximum(d2, 1e-12)).min(axis=2)
    else:
        loss_dist = np.linalg.norm(ppred - tgt, axis=2)
    c = np.clip(pred_conf[..., 0].astype(np.float64), 1e-4, 1.0)
    return np.float32(np.mean(loss_dist * c - W_RATE * np.log(c)))


def kernel(pred_quat, pred_trans, pred_conf, pose, points, cls_id):
    pred_quat = _np_f32(pred_quat)
    pred_trans = _np_f32(pred_trans)
    pred_conf = _np_f32(pred_conf)
    pose = _np_f32(pose)
    points = _np_f32(points)
    cls_id = np.asarray(cls_id)

    assert pred_quat.shape == (B, N, 4), pred_quat.shape

    if int(cls_id[0]) not in SYM_CLASS_IDS:
        return np.array(
            _numpy_reference(pred_quat, pred_trans, pred_conf, pose, points,
                             cls_id),
            dtype=np.float32)

    from concourse.bass_utils import run_bass_kernel_spmd

    nc = _get_nc()
    in_maps = [
        {
            "pred_quat": np.ascontiguousarray(pred_quat[c]),
            "pred_trans": np.ascontiguousarray(pred_trans[c]),
            "pred_conf": np.ascontiguousarray(pred_conf[c]),
            "pose": np.ascontiguousarray(pose[c]),
            "points": np.ascontiguousarray(points[c]),
        }
        for c in range(B)
    ]
    res = run_bass_kernel_spmd(nc, in_maps, core_ids=list(range(B)))
    total = np.float64(0.0)
    for r in res.results:
        total += np.sum(r["out_sums"].astype(np.float64))
    return np.array(total / (B * N), dtype=np.float32)
